# revision 22
# baseline (speedup 1.0000x reference)
"""AliasFreeActivation Trainium2 kernel.

out = D @ lrelu(U X U^T) @ D^T per channel, where U (336x84) is the x4
upsample band matrix, D (148x336) the x2 downsample band matrix with the
output crop folded in.  Only columns [15, 321) of D are nonzero, so the
intermediate Y = U X U^T is computed on the 306x306 subgrid.

Mapping to the PE (out = lhsT.T @ rhs, contraction over partitions):
  S1: o1   = PE(X,  U')          [84(c),   306(r')]     (V-up)
  S2: Y_w  = PE(o1[:,w:w+128], U')  3 r'-windows -> [128, 306(n')]  (H-up)
      drain with fused leaky-relu on ScalarE (ACT), cast to bf16
  S3: C1_w[:, dg] = PE(Y_g[:, w:w+128], D'_g)   9 matmuls, no K-accum
  S4: Z[dchunk, d2g] = PE(C1_g'[(ch,d) chunk], D'_g')   4-channel batched
Each down-filter output group g only needs one 128-row window of its
input (band structure), so S3/S4 are single matmuls with zero-padded
weight blocks D'_g instead of 3-way K accumulations.

All matmul operands are bf16 (fp32 runs the PE at 1/4 rate via
FP32_HIGH/LOW; float32r lowers to the same path).  PSUM accumulates f32;
end-to-end relative error vs the f32 reference is ~2.4e-3.

Schedule shape (per core: 64 channels, ~20 matmuls/channel):
 - S1 is emitted for channel pairs sharing one 2-bank PSUM tile so the
   o1 drain is a single DVE op per pair.
 - S2's six N=153 matmuls pack the three Y windows into 2 PSUM banks
   (3 x 612B per bank); each bank is an independent pool tile drained by
   one fused Prelu ACTIVATE, so banks recycle as soon as they drain.
 - Y/C1/Z drains are split DVE/ACT to balance the two PSUM-read engines.
 - PSUM budget (8 banks): S1-pair 2 + Y 4 + down(S3/S4 shared) 2.
Measured on trn2 (PE clock capped at 1.2 GHz by the platform power
profile): ~110 us/core HW exec for the full 512-channel problem.
"""
import numpy as np
import ml_dtypes
from contextlib import ExitStack

# ---- problem geometry (hardcoded; matches nn_AliasFreeActivation) ----
BATCH, CH, HIN = 2, 256, 84
HOUT = 148
N_CORES = 8
CH_PER_CORE = BATCH * CH // N_CORES     # 64
LO, HI = 15, 321                        # needed Y range (D band support)
R = HI - LO                             # 306
WINDOWS = (0, 89, 178)                  # r'/n' window starts
WIDTHS = (128, 128, 128)                # window widths (128 keeps fast LDW)
DGROUPS = ((0, 59), (59, 103), (103, 148))  # output cols per window
GROUP4 = 4                              # channels batched in S4
NCHUNK4 = (GROUP4 * HOUT + 127) // 128  # 5 M-chunks per 4 channels

_BF16 = ml_dtypes.bfloat16


def _build_U(up):
    """336x84 band matrix of upfirdn(up=4, pad=(13,13), true conv)."""
    U = np.zeros((336, 84), dtype=np.float64)
    kf = np.asarray(up, np.float64)[::-1]
    for o in range(336):
        for t in range(24):
            j = o + t - 13
            if j >= 0 and j % 4 == 0 and j // 4 < 84:
                U[o, j // 4] += kf[t]
    return U


def _build_D(down):
    """148x336: stride-2 12-tap conv (pad 5/5) with rows 10:158 (crop)."""
    Df = np.zeros((168, 336), dtype=np.float64)
    kf = np.asarray(down, np.float64)[::-1]
    for o in range(168):
        for t in range(12):
            j = 2 * o + t - 5
            if 0 <= j < 336:
                Df[o, j] += kf[t]
    return Df[10:158]


def _build_program():
    """Build the per-core Bass program (identical for all 8 cores)."""
    from concourse import bacc
    import concourse.tile as tile
    import concourse.mybir as mybir

    bf16 = mybir.dt.bfloat16
    f32 = mybir.dt.float32
    f32r = mybir.dt.float32r

    nc = bacc.Bacc("TRN2", target_bir_lowering=False, debug=False, num_devices=1)
    x_d = nc.dram_tensor("x", [CH_PER_CORE, HIN, HIN], bf16, kind="ExternalInput").ap()
    up_d = nc.dram_tensor("up", [HIN, R], bf16, kind="ExternalInput").ap()
    dp_d = nc.dram_tensor("dp", [128, HOUT], bf16, kind="ExternalInput").ap()
    out_d = nc.dram_tensor("out", [CH_PER_CORE, HOUT, HOUT], f32,
                           kind="ExternalOutput").ap()
    out_flat = out_d.rearrange("a b c -> (a b) c")

    with tile.TileContext(nc) as tc, ExitStack() as ctx:
        cpool = ctx.enter_context(tc.tile_pool(name="consts15", bufs=1))
        px = ctx.enter_context(tc.tile_pool(name="x", bufs=6))
        po1 = ctx.enter_context(tc.tile_pool(name="o1", bufs=6))
        py = ctx.enter_context(tc.tile_pool(name="y", bufs=5))
        pc1 = ctx.enter_context(tc.tile_pool(name="c1", bufs=4))
        ps4 = ctx.enter_context(tc.tile_pool(name="s4", bufs=4))
        pp1 = ctx.enter_context(tc.tile_pool(name="pp1", bufs=1, space="PSUM"))
        ppy = ctx.enter_context(tc.tile_pool(name="ppy", bufs=4, space="PSUM"))
        ppd = ctx.enter_context(tc.tile_pool(name="ppd", bufs=2, space="PSUM"))

        up_sb = cpool.tile([HIN, R], bf16)
        nc.sync.dma_start(up_sb[:], up_d[:])
        # trigger the Prelu act-table load before the pipeline needs it
        warm = cpool.tile([1, 2], f32)
        nc.gpsimd.memset(warm[:], 0.0)
        warm2 = cpool.tile([1, 2], f32)
        nc.scalar.activation(warm2[:], warm[:],
                             mybir.ActivationFunctionType.Prelu, alpha=0.2)
        dp_sb = cpool.tile([128, HOUT], bf16)
        nc.sync.dma_start(dp_sb[:], dp_d[:])

        for q in range(CH_PER_CORE // GROUP4):
            # batched input DMA: [84(r), 4(ch), 84(c)]
            x_sb = px.tile([HIN, GROUP4, HIN], bf16)
            nc.gpsimd.dma_start(
                x_sb[:], x_d[q * GROUP4:(q + 1) * GROUP4].rearrange("ch r c -> r ch c"))

            c1_sb = pc1.tile([128, 3, GROUP4, HOUT], bf16)
            for pair in range(GROUP4 // 2):
                # S1 for a channel pair -> one PSUM tile, one drain
                p1 = pp1.tile([HIN, 2, 512], f32)
                for e in range(2):
                    nc.tensor.matmul(p1[:, e, :R], x_sb[:, 2 * pair + e, :],
                                     up_sb[:])
                o1_sb = po1.tile([HIN, 2 * R], bf16)
                nc.vector.tensor_copy(
                    o1_sb[:].rearrange("p (e r) -> p e r", e=2), p1[:, :, :R])

                for e in range(2):
                    # S2: six N=153 half-window matmuls; each PSUM bank is
                    # its own pool tile so a drained bank recycles at once.
                    y_sb = py.tile([128, 3 * R], bf16)
                    for b in range(2):
                        ypb = ppy.tile([128, 512], f32)
                        for j in range(3):
                            c = 3 * b + j
                            w, h = c // 2, c % 2
                            nc.tensor.matmul(
                                ypb[:, 153 * j:153 * (j + 1)],
                                o1_sb[:, R * e + WINDOWS[w]:
                                      R * e + WINDOWS[w] + 128],
                                up_sb[:, 153 * h:153 * h + 153])
                        nc.scalar.activation(
                            y_sb[:, 459 * b:459 * (b + 1)].rearrange(
                                "p (a s) -> p a s", s=153),
                            ypb[:, :459].rearrange("p (a s) -> p a s", s=153),
                            mybir.ActivationFunctionType.Prelu, alpha=0.2)

                    # S3: C1_w[:, d-group g] from Y window g  (bf16)
                    cps = ppd.tile([128, 3, HOUT], f32, tag="dpsum")
                    for wi in range(3):
                        for g, (d0, d1) in enumerate(DGROUPS):
                            nc.tensor.matmul(
                                cps[:WIDTHS[wi], wi, d0:d1],
                                y_sb[:WIDTHS[g],
                                     306 * g + WINDOWS[wi]:
                                     306 * g + WINDOWS[wi] + WIDTHS[wi]],
                                dp_sb[:WIDTHS[g], d0:d1])
                    nc.vector.tensor_copy(c1_sb[:, :, 2 * pair + e, :], cps[:])

            # S4: 4-channel batched over (ch, d) chunks of 128
            c1_flat = c1_sb[:].rearrange("p a b c -> p a (b c)")
            z_sb = ps4.tile([128, NCHUNK4, HOUT], f32)
            for k in range(NCHUNK4):
                m0 = 128 * k
                m1 = min(m0 + 128, GROUP4 * HOUT)
                sps = ppd.tile([128, HOUT], f32, tag="dpsum")
                for gp, (e0, e1) in enumerate(DGROUPS):
                    nc.tensor.matmul(sps[:m1 - m0, e0:e1],
                                     c1_flat[:WIDTHS[gp], gp, m0:m1],
                                     dp_sb[:WIDTHS[gp], e0:e1])
                nc.vector.tensor_copy(z_sb[:m1 - m0, k, :], sps[:m1 - m0, :])
            # 2 output DMAs per 4 channels (512 rows + 80 rows)
            r0 = q * GROUP4 * HOUT
            dst = out_flat[r0:r0 + 512].rearrange("(j p) c -> p j c", p=128)
            nc.sync.dma_start(dst, z_sb[:, :4, :])
            nc.sync.dma_start(out_flat[r0 + 512:r0 + 592], z_sb[:80, 4, :])

    nc.compile()
    return nc


def _prep_inputs(x, up_filter, down_filter):
    """Host-side: band matrices, casts, shards. Returns per-core in_maps."""
    U = _build_U(np.asarray(up_filter, np.float64))
    D = _build_D(np.asarray(down_filter, np.float64))
    Up = np.ascontiguousarray(U[LO:HI, :].T)          # [84, 306]
    Dp = np.zeros((128, HOUT))
    for g, (d0, d1) in enumerate(DGROUPS):
        for k in range(128):
            col = WINDOWS[g] + k
            if col < R:
                Dp[k, d0:d1] = D[d0:d1, LO + col]

    xf = np.asarray(x).reshape(BATCH * CH, HIN, HIN).astype(_BF16)
    up_f = Up.astype(_BF16)
    dp_b = Dp.astype(_BF16)
    in_maps = []
    for c in range(N_CORES):
        shard = np.ascontiguousarray(xf[c * CH_PER_CORE:(c + 1) * CH_PER_CORE])
        in_maps.append({"x": shard, "up": up_f, "dp": dp_b})
    return in_maps


def kernel(x, up_filter, down_filter, _trace=False, _perf_out=None):
    import concourse.bass_utils as _bu

    nc = _build_program()
    in_maps = _prep_inputs(x, up_filter, down_filter)
    res = _bu.run_bass_kernel_spmd(nc, in_maps, core_ids=list(range(N_CORES)),
                                   trace=_trace)
    if _perf_out is not None:
        _perf_out["exec_time_ns"] = res.exec_time_ns
        _perf_out["results"] = res
    out = np.empty((BATCH * CH, HOUT, HOUT), np.float32)
    for c in range(N_CORES):
        out[c * CH_PER_CORE:(c + 1) * CH_PER_CORE] = res.results[c]["out"]
    return out.reshape(BATCH, CH, HOUT, HOUT)


# revision 23
# speedup vs baseline: 1.3394x; 1.3394x over previous
"""AliasFreeActivation Trainium2 kernel.

out = D @ lrelu(U X U^T) @ D^T per channel, where U (336x84) is the x4
upsample band matrix, D (148x336) the x2 downsample band matrix with the
output crop folded in.  Only columns [15, 321) of D are nonzero, so the
intermediate Y = U X U^T is computed on the 306x306 subgrid.

Mapping to the PE (out = lhsT.T @ rhs, contraction over partitions):
  S1: o1   = PE(X,  U')          [84(c),   306(r')]     (V-up)
  S2: Y_w  = PE(o1[:,w:w+128], U')  3 r'-windows -> [128, 306(n')]  (H-up)
      drain with fused leaky-relu on ScalarE (ACT), cast to bf16
  S3: C1_w[:, dg] = PE(Y_g[:, w:w+128], D'_g)   9 matmuls, no K-accum
  S4: Z[dchunk, d2g] = PE(C1_g'[(ch,d) chunk], D'_g')   4-channel batched
Each down-filter output group g only needs one 128-row window of its
input (band structure), so S3/S4 are single matmuls with zero-padded
weight blocks D'_g instead of 3-way K accumulations.

All matmul operands are bf16 (fp32 runs the PE at 1/4 rate via
FP32_HIGH/LOW; float32r lowers to the same path).  PSUM accumulates f32;
end-to-end relative error vs the f32 reference is ~2.4e-3.

Schedule shape (per core: 64 channels, ~20 matmuls/channel):
 - S1 is emitted for channel pairs sharing one 2-bank PSUM tile so the
   o1 drain is a single DVE op per pair.
 - S2's six N=153 matmuls pack the three Y windows into 2 PSUM banks
   (3 x 612B per bank); each bank is an independent pool tile drained by
   one fused Prelu ACTIVATE, so banks recycle as soon as they drain.
 - Y drains run on ACT, o1/C1 drains on DVE, Z drains alternate.
 - PSUM budget (8 banks): S1-pair 2 + Y 4 + down(S3/S4 shared) 2.
Measured on trn2 (PE clock capped at 1.2 GHz by the platform power
profile): ~110 us/core HW exec for the full 512-channel problem.
"""
import numpy as np
import ml_dtypes
from contextlib import ExitStack

# ---- problem geometry (hardcoded; matches nn_AliasFreeActivation) ----
BATCH, CH, HIN = 2, 256, 84
HOUT = 148
N_CORES = 8
CH_PER_CORE = BATCH * CH // N_CORES     # 64
LO, HI = 15, 321                        # needed Y range (D band support)
R = HI - LO                             # 306
WINDOWS = (0, 89, 178)                  # r'/n' window starts
WIDTHS = (128, 128, 128)                # window widths (128 keeps fast LDW)
DGROUPS = ((0, 59), (59, 103), (103, 148))  # output cols per window
GROUP4 = 4                              # channels batched in S4
NCHUNK4 = (GROUP4 * HOUT + 127) // 128  # 5 M-chunks per 4 channels

_BF16 = ml_dtypes.bfloat16


def _build_U(up):
    """336x84 band matrix of upfirdn(up=4, pad=(13,13), true conv)."""
    U = np.zeros((336, 84), dtype=np.float64)
    kf = np.asarray(up, np.float64)[::-1]
    for o in range(336):
        for t in range(24):
            j = o + t - 13
            if j >= 0 and j % 4 == 0 and j // 4 < 84:
                U[o, j // 4] += kf[t]
    return U


def _build_D(down):
    """148x336: stride-2 12-tap conv (pad 5/5) with rows 10:158 (crop)."""
    Df = np.zeros((168, 336), dtype=np.float64)
    kf = np.asarray(down, np.float64)[::-1]
    for o in range(168):
        for t in range(12):
            j = 2 * o + t - 5
            if 0 <= j < 336:
                Df[o, j] += kf[t]
    return Df[10:158]


def _build_program():
    """Build the per-core Bass program (identical for all 8 cores)."""
    from concourse import bacc
    import concourse.tile as tile
    import concourse.mybir as mybir

    bf16 = mybir.dt.bfloat16
    f32 = mybir.dt.float32
    f32r = mybir.dt.float32r

    nc = bacc.Bacc("TRN2", target_bir_lowering=False, debug=False, num_devices=1)
    x_d = nc.dram_tensor("x", [CH_PER_CORE, HIN, HIN], bf16, kind="ExternalInput").ap()
    up_d = nc.dram_tensor("up", [HIN, R], bf16, kind="ExternalInput").ap()
    dp_d = nc.dram_tensor("dp", [128, HOUT], bf16, kind="ExternalInput").ap()
    out_d = nc.dram_tensor("out", [CH_PER_CORE, HOUT, HOUT], f32,
                           kind="ExternalOutput").ap()
    out_flat = out_d.rearrange("a b c -> (a b) c")

    with tile.TileContext(nc) as tc, ExitStack() as ctx:
        cpool = ctx.enter_context(tc.tile_pool(name="consts13", bufs=1))
        px = ctx.enter_context(tc.tile_pool(name="x", bufs=6))
        po1 = ctx.enter_context(tc.tile_pool(name="o1", bufs=6))
        py = ctx.enter_context(tc.tile_pool(name="y", bufs=5))
        pc1 = ctx.enter_context(tc.tile_pool(name="c1", bufs=4))
        ps4 = ctx.enter_context(tc.tile_pool(name="s4", bufs=4))
        pp1 = ctx.enter_context(tc.tile_pool(name="pp1", bufs=1, space="PSUM"))
        ppy = ctx.enter_context(tc.tile_pool(name="ppy", bufs=4, space="PSUM"))
        ppd = ctx.enter_context(tc.tile_pool(name="ppd", bufs=2, space="PSUM"))

        up_sb = cpool.tile([HIN, R], bf16)
        nc.sync.dma_start(up_sb[:], up_d[:])
        dp_sb = cpool.tile([128, HOUT], bf16)
        nc.sync.dma_start(dp_sb[:], dp_d[:])

        for q in range(CH_PER_CORE // GROUP4):
            # batched input DMA: [84(r), 4(ch), 84(c)]
            x_sb = px.tile([HIN, GROUP4, HIN], bf16)
            nc.gpsimd.dma_start(
                x_sb[:], x_d[q * GROUP4:(q + 1) * GROUP4].rearrange("ch r c -> r ch c"))

            c1_sb = pc1.tile([128, 3, GROUP4, HOUT], bf16)
            for pair in range(GROUP4 // 2):
                # S1 for a channel pair -> one PSUM tile, one drain
                p1 = pp1.tile([HIN, 2, 512], f32)
                for e in range(2):
                    nc.tensor.matmul(p1[:, e, :R], x_sb[:, 2 * pair + e, :],
                                     up_sb[:])
                o1_sb = po1.tile([HIN, 2 * R], bf16)
                nc.vector.tensor_copy(
                    o1_sb[:].rearrange("p (e r) -> p e r", e=2), p1[:, :, :R])

                for e in range(2):
                    # S2: six N=153 half-window matmuls; each PSUM bank is
                    # its own pool tile so a drained bank recycles at once.
                    y_sb = py.tile([128, 3 * R], bf16)
                    for b in range(2):
                        ypb = ppy.tile([128, 512], f32)
                        for j in range(3):
                            c = 3 * b + j
                            w, h = c // 2, c % 2
                            nc.tensor.matmul(
                                ypb[:, 153 * j:153 * (j + 1)],
                                o1_sb[:, R * e + WINDOWS[w]:
                                      R * e + WINDOWS[w] + 128],
                                up_sb[:, 153 * h:153 * h + 153])
                        nc.scalar.activation(
                            y_sb[:, 459 * b:459 * (b + 1)].rearrange(
                                "p (a s) -> p a s", s=153),
                            ypb[:, :459].rearrange("p (a s) -> p a s", s=153),
                            mybir.ActivationFunctionType.Prelu, alpha=0.2)

                    # S3: C1_w[:, d-group g] from Y window g  (bf16)
                    cps = ppd.tile([128, 3, HOUT], f32, tag="dpsum")
                    for wi in range(3):
                        for g, (d0, d1) in enumerate(DGROUPS):
                            nc.tensor.matmul(
                                cps[:WIDTHS[wi], wi, d0:d1],
                                y_sb[:WIDTHS[g],
                                     306 * g + WINDOWS[wi]:
                                     306 * g + WINDOWS[wi] + WIDTHS[wi]],
                                dp_sb[:WIDTHS[g], d0:d1])
                    nc.vector.tensor_copy(c1_sb[:, :, 2 * pair + e, :], cps[:])

            # S4: 4-channel batched over (ch, d) chunks of 128
            c1_flat = c1_sb[:].rearrange("p a b c -> p a (b c)")
            z_sb = ps4.tile([128, NCHUNK4, HOUT], f32)
            for k in range(NCHUNK4):
                m0 = 128 * k
                m1 = min(m0 + 128, GROUP4 * HOUT)
                sps = ppd.tile([128, HOUT], f32, tag="dpsum")
                for gp, (e0, e1) in enumerate(DGROUPS):
                    nc.tensor.matmul(sps[:m1 - m0, e0:e1],
                                     c1_flat[:WIDTHS[gp], gp, m0:m1],
                                     dp_sb[:WIDTHS[gp], e0:e1])
                if k % 2 == 0:
                    nc.vector.tensor_copy(z_sb[:m1 - m0, k, :], sps[:m1 - m0, :])
                else:
                    nc.scalar.activation(z_sb[:m1 - m0, k, :], sps[:m1 - m0, :],
                                         mybir.ActivationFunctionType.Copy)
            # 2 output DMAs per 4 channels (512 rows + 80 rows)
            r0 = q * GROUP4 * HOUT
            dst = out_flat[r0:r0 + 512].rearrange("(j p) c -> p j c", p=128)
            nc.sync.dma_start(dst, z_sb[:, :4, :])
            nc.sync.dma_start(out_flat[r0 + 512:r0 + 592], z_sb[:80, 4, :])

    nc.compile()
    return nc


def _prep_inputs(x, up_filter, down_filter):
    """Host-side: band matrices, casts, shards. Returns per-core in_maps."""
    U = _build_U(np.asarray(up_filter, np.float64))
    D = _build_D(np.asarray(down_filter, np.float64))
    Up = np.ascontiguousarray(U[LO:HI, :].T)          # [84, 306]
    Dp = np.zeros((128, HOUT))
    for g, (d0, d1) in enumerate(DGROUPS):
        for k in range(128):
            col = WINDOWS[g] + k
            if col < R:
                Dp[k, d0:d1] = D[d0:d1, LO + col]

    xf = np.asarray(x).reshape(BATCH * CH, HIN, HIN).astype(_BF16)
    up_f = Up.astype(_BF16)
    dp_b = Dp.astype(_BF16)
    in_maps = []
    for c in range(N_CORES):
        shard = np.ascontiguousarray(xf[c * CH_PER_CORE:(c + 1) * CH_PER_CORE])
        in_maps.append({"x": shard, "up": up_f, "dp": dp_b})
    return in_maps


def kernel(x, up_filter, down_filter, _trace=False, _perf_out=None):
    import concourse.bass_utils as _bu

    nc = _build_program()
    in_maps = _prep_inputs(x, up_filter, down_filter)
    res = _bu.run_bass_kernel_spmd(nc, in_maps, core_ids=list(range(N_CORES)),
                                   trace=_trace)
    if _perf_out is not None:
        _perf_out["exec_time_ns"] = res.exec_time_ns
        _perf_out["results"] = res
    out = np.empty((BATCH * CH, HOUT, HOUT), np.float32)
    for c in range(N_CORES):
        out[c * CH_PER_CORE:(c + 1) * CH_PER_CORE] = res.results[c]["out"]
    return out.reshape(BATCH, CH, HOUT, HOUT)


# revision 24
# speedup vs baseline: 1.3442x; 1.0036x over previous
"""AliasFreeActivation Trainium2 kernel.

out = D @ lrelu(U X U^T) @ D^T per channel, where U (336x84) is the x4
upsample band matrix, D (148x336) the x2 downsample band matrix with the
output crop folded in.  Only columns [15, 321) of D are nonzero, so the
intermediate Y = U X U^T is computed on the 306x306 subgrid.

Mapping to the PE (out = lhsT.T @ rhs, contraction over partitions):
  S1: o1   = PE(X,  U')          [84(c),   306(r')]     (V-up)
  S2: Y_w  = PE(o1[:,w:w+128], U')  3 r'-windows -> [128, 306(n')]  (H-up)
      drain with fused leaky-relu on ScalarE (ACT), cast to bf16
  S3: C1_w[:, dg] = PE(Y_g[:, w:w+128], D'_g)   9 matmuls, no K-accum
  S4: Z[dchunk, d2g] = PE(C1_g'[(ch,d) chunk], D'_g')   4-channel batched
Each down-filter output group g only needs one 128-row window of its
input (band structure), so S3/S4 are single matmuls with zero-padded
weight blocks D'_g instead of 3-way K accumulations.

All matmul operands are bf16 (fp32 runs the PE at 1/4 rate via
FP32_HIGH/LOW; float32r lowers to the same path).  PSUM accumulates f32;
end-to-end relative error vs the f32 reference is ~2.4e-3.

Schedule shape (per core: 64 channels, ~20 matmuls/channel):
 - S1 is emitted for channel pairs sharing one 2-bank PSUM tile so the
   o1 drain is a single DVE op per pair.
 - S2's six N=153 matmuls pack the three Y windows into 2 PSUM banks
   (3 x 612B per bank); each bank is an independent pool tile drained by
   one fused Prelu ACTIVATE, so banks recycle as soon as they drain.
 - Y drains run on ACT, o1/C1 drains on DVE, Z drains alternate.
 - PSUM budget (8 banks): S1-pair 2 + Y 4 + down(S3/S4 shared) 2.
Measured on trn2 (PE clock capped at 1.2 GHz by the platform power
profile): ~110 us/core HW exec for the full 512-channel problem.
"""
import numpy as np
import ml_dtypes
from contextlib import ExitStack

# ---- problem geometry (hardcoded; matches nn_AliasFreeActivation) ----
BATCH, CH, HIN = 2, 256, 84
HOUT = 148
N_CORES = 8
CH_PER_CORE = BATCH * CH // N_CORES     # 64
LO, HI = 15, 321                        # needed Y range (D band support)
R = HI - LO                             # 306
WINDOWS = (0, 89, 178)                  # r'/n' window starts
WIDTHS = (128, 128, 128)                # window widths (128 keeps fast LDW)
DGROUPS = ((0, 59), (59, 103), (103, 148))  # output cols per window
GROUP4 = 4                              # channels batched in S4
NCHUNK4 = (GROUP4 * HOUT + 127) // 128  # 5 M-chunks per 4 channels

_BF16 = ml_dtypes.bfloat16


def _build_U(up):
    """336x84 band matrix of upfirdn(up=4, pad=(13,13), true conv)."""
    U = np.zeros((336, 84), dtype=np.float64)
    kf = np.asarray(up, np.float64)[::-1]
    for o in range(336):
        for t in range(24):
            j = o + t - 13
            if j >= 0 and j % 4 == 0 and j // 4 < 84:
                U[o, j // 4] += kf[t]
    return U


def _build_D(down):
    """148x336: stride-2 12-tap conv (pad 5/5) with rows 10:158 (crop)."""
    Df = np.zeros((168, 336), dtype=np.float64)
    kf = np.asarray(down, np.float64)[::-1]
    for o in range(168):
        for t in range(12):
            j = 2 * o + t - 5
            if 0 <= j < 336:
                Df[o, j] += kf[t]
    return Df[10:158]


def _build_program():
    """Build the per-core Bass program (identical for all 8 cores)."""
    from concourse import bacc
    import concourse.tile as tile
    import concourse.mybir as mybir

    bf16 = mybir.dt.bfloat16
    f32 = mybir.dt.float32
    f32r = mybir.dt.float32r

    nc = bacc.Bacc("TRN2", target_bir_lowering=False, debug=False, num_devices=1)
    x_d = nc.dram_tensor("x", [CH_PER_CORE, HIN, HIN], bf16, kind="ExternalInput").ap()
    up_d = nc.dram_tensor("up", [HIN, R], bf16, kind="ExternalInput").ap()
    dp_d = nc.dram_tensor("dp", [128, HOUT], bf16, kind="ExternalInput").ap()
    out_d = nc.dram_tensor("out", [CH_PER_CORE, HOUT, HOUT], f32,
                           kind="ExternalOutput").ap()
    out_flat = out_d.rearrange("a b c -> (a b) c")

    with tile.TileContext(nc) as tc, ExitStack() as ctx:
        cpool = ctx.enter_context(tc.tile_pool(name="consts16", bufs=1))
        px = ctx.enter_context(tc.tile_pool(name="x", bufs=6))
        po1 = ctx.enter_context(tc.tile_pool(name="o1", bufs=6))
        py = ctx.enter_context(tc.tile_pool(name="y", bufs=5))
        pc1 = ctx.enter_context(tc.tile_pool(name="c1", bufs=4))
        ps4 = ctx.enter_context(tc.tile_pool(name="s4", bufs=4))
        pp1 = ctx.enter_context(tc.tile_pool(name="pp1", bufs=1, space="PSUM"))
        ppy = ctx.enter_context(tc.tile_pool(name="ppy", bufs=4, space="PSUM"))
        ppd = ctx.enter_context(tc.tile_pool(name="ppd", bufs=2, space="PSUM"))

        up_sb = cpool.tile([HIN, R], bf16)
        nc.sync.dma_start(up_sb[:], up_d[:])
        dp_sb = cpool.tile([128, HOUT], bf16)
        nc.sync.dma_start(dp_sb[:], dp_d[:])

        for q in range(CH_PER_CORE // GROUP4):
            # batched input DMA: [84(r), 4(ch), 84(c)]
            x_sb = px.tile([HIN, GROUP4, HIN], bf16)
            nc.gpsimd.dma_start(
                x_sb[:], x_d[q * GROUP4:(q + 1) * GROUP4].rearrange("ch r c -> r ch c"))

            c1_sb = pc1.tile([128, 3, GROUP4, HOUT], bf16)
            for pair in range(GROUP4 // 2):
                # S1 for a channel pair -> one PSUM tile, one drain
                p1 = pp1.tile([HIN, 2, 512], f32)
                for e in range(2):
                    nc.tensor.matmul(p1[:, e, :R], x_sb[:, 2 * pair + e, :],
                                     up_sb[:])
                o1_sb = po1.tile([HIN, 2 * R], bf16)
                nc.vector.tensor_copy(
                    o1_sb[:].rearrange("p (e r) -> p e r", e=2), p1[:, :, :R])

                for e in range(2):
                    # S2: six N=153 half-window matmuls; each PSUM bank is
                    # its own pool tile so a drained bank recycles at once.
                    y_sb = py.tile([128, 3 * R], bf16)
                    for b in range(2):
                        ypb = ppy.tile([128, 512], f32)
                        for j in range(3):
                            c = 3 * b + j
                            w, h = c // 2, c % 2
                            nc.tensor.matmul(
                                ypb[:, 153 * j:153 * (j + 1)],
                                o1_sb[:, R * e + WINDOWS[w]:
                                      R * e + WINDOWS[w] + 128],
                                up_sb[:, 153 * h:153 * h + 153])
                        nc.scalar.activation(
                            y_sb[:, 459 * b:459 * (b + 1)], ypb[:, :459],
                            mybir.ActivationFunctionType.Prelu, alpha=0.2)

                    # S3: C1_w[:, d-group g] from Y window g  (bf16)
                    cps = ppd.tile([128, 3, HOUT], f32, tag="dpsum")
                    for wi in range(3):
                        for g, (d0, d1) in enumerate(DGROUPS):
                            nc.tensor.matmul(
                                cps[:WIDTHS[wi], wi, d0:d1],
                                y_sb[:WIDTHS[g],
                                     306 * g + WINDOWS[wi]:
                                     306 * g + WINDOWS[wi] + WIDTHS[wi]],
                                dp_sb[:WIDTHS[g], d0:d1])
                    nc.vector.tensor_copy(c1_sb[:, :, 2 * pair + e, :], cps[:])

            # S4: 4-channel batched over (ch, d) chunks of 128
            c1_flat = c1_sb[:].rearrange("p a b c -> p a (b c)")
            z_sb = ps4.tile([128, NCHUNK4, HOUT], f32)
            for k in range(NCHUNK4):
                m0 = 128 * k
                m1 = min(m0 + 128, GROUP4 * HOUT)
                sps = ppd.tile([128, HOUT], f32, tag="dpsum")
                for gp, (e0, e1) in enumerate(DGROUPS):
                    nc.tensor.matmul(sps[:m1 - m0, e0:e1],
                                     c1_flat[:WIDTHS[gp], gp, m0:m1],
                                     dp_sb[:WIDTHS[gp], e0:e1])
                if k % 2 == 0:
                    nc.vector.tensor_copy(z_sb[:m1 - m0, k, :], sps[:m1 - m0, :])
                else:
                    nc.scalar.activation(z_sb[:m1 - m0, k, :], sps[:m1 - m0, :],
                                         mybir.ActivationFunctionType.Copy)
            # 2 output DMAs per 4 channels (512 rows + 80 rows)
            r0 = q * GROUP4 * HOUT
            dst = out_flat[r0:r0 + 512].rearrange("(j p) c -> p j c", p=128)
            nc.sync.dma_start(dst, z_sb[:, :4, :])
            nc.sync.dma_start(out_flat[r0 + 512:r0 + 592], z_sb[:80, 4, :])

    nc.compile()
    return nc


def _prep_inputs(x, up_filter, down_filter):
    """Host-side: band matrices, casts, shards. Returns per-core in_maps."""
    U = _build_U(np.asarray(up_filter, np.float64))
    D = _build_D(np.asarray(down_filter, np.float64))
    Up = np.ascontiguousarray(U[LO:HI, :].T)          # [84, 306]
    Dp = np.zeros((128, HOUT))
    for g, (d0, d1) in enumerate(DGROUPS):
        for k in range(128):
            col = WINDOWS[g] + k
            if col < R:
                Dp[k, d0:d1] = D[d0:d1, LO + col]

    xf = np.asarray(x).reshape(BATCH * CH, HIN, HIN).astype(_BF16)
    up_f = Up.astype(_BF16)
    dp_b = Dp.astype(_BF16)
    in_maps = []
    for c in range(N_CORES):
        shard = np.ascontiguousarray(xf[c * CH_PER_CORE:(c + 1) * CH_PER_CORE])
        in_maps.append({"x": shard, "up": up_f, "dp": dp_b})
    return in_maps


def kernel(x, up_filter, down_filter, _trace=False, _perf_out=None):
    import concourse.bass_utils as _bu

    nc = _build_program()
    in_maps = _prep_inputs(x, up_filter, down_filter)
    res = _bu.run_bass_kernel_spmd(nc, in_maps, core_ids=list(range(N_CORES)),
                                   trace=_trace)
    if _perf_out is not None:
        _perf_out["exec_time_ns"] = res.exec_time_ns
        _perf_out["results"] = res
    out = np.empty((BATCH * CH, HOUT, HOUT), np.float32)
    for c in range(N_CORES):
        out[c * CH_PER_CORE:(c + 1) * CH_PER_CORE] = res.results[c]["out"]
    return out.reshape(BATCH, CH, HOUT, HOUT)
